# revision 8
# baseline (speedup 1.0000x reference)
"""Self-contained Trainium2 Bass kernel for nn_CA_9363028705415 (sparse_attention).

Computes, per batch b:
    Q = relu(x[b] @ qW1 + qb1) @ qW2 + qb2          # [M, K]
    Kt = relu(x[b] @ kW1 + kb1) @ kW2 + kb2         # [M, K]
    S = Q @ Kt.T                                    # [M, M]
    out[b] = softmax(S / rowmax(S), axis=-1)        # max-DIVISION normalization

Shapes: B=16, M=2048, D=128, H=256, K=64.  Output [16, 2048, 2048] f32 (256 MB)
=> memory-bound on the output write (~32 MB/core across 8 cores).

Sharding: data-parallel over batch across 8 NeuronCores; 2 batches/core; tiny
MLP weights replicated.  Single NEFF run SPMD via run_bass_kernel_spmd.
"""

import numpy as np

import concourse.bass as bass
import concourse.mybir as mybir
from concourse import bacc
import concourse.tile as tile
from concourse.bass import ts
from concourse.bass_utils import run_bass_kernel_spmd
from concourse.masks import make_identity

F32 = mybir.dt.float32
BF16 = mybir.dt.bfloat16
AF = mybir.ActivationFunctionType
ALU = mybir.AluOpType

N_CORES = 8
B, M, D, H, KF = 16, 2048, 128, 256, 64
BPC = B // N_CORES     # batches per core
MT = M // 128          # 16 row-tiles per batch
FC = M // 512          # 4 matmul free-chunks of 512
PAIR = 2               # row-tiles per output DMA (2 MB chunks)

# Engine assignment knobs (tuned from profiling).
# normalize engine per row-tile, cycled: DVE tensor_scalar is 2x fp32 SBUF
# (1.28us/tile), ACT copy-with-scale is 1x (2.06us/tile).  gpsimd is NOT used:
# its software tensor_scalar is ~29us/tile AND locks the DVE-shared SBUF port.
NORM_PATTERN = (
    "dve", "act", "dve", "act", "dve", "dve", "dve", "act",
    "dve", "act", "dve", "dve", "dve", "act", "dve", "dve",
)
RELU_ENGINES = ("act", "act", "act", "act")  # hT evac per (head, pc)
QT_EVAC = "dve"
KT_EVAC = "act"
XT_EVAC = "dve"


def _evac_bias(nc, engine, out, in_, bias, relu):
    """out = [relu](in_ + bias), bias is [P,1] per-partition AP."""
    if engine == "act":
        nc.scalar.activation(
            out, in_, AF.Relu if relu else AF.Identity, bias=bias, scale=1.0
        )
    else:
        eng = nc.vector if engine == "dve" else nc.gpsimd
        if relu:
            eng.tensor_scalar(
                out, in_, bias, 0.0, op0=ALU.add, op1=ALU.max
            )
        else:
            eng.tensor_scalar(out, in_, bias, None, op0=ALU.add)


def _norm(nc, engine, out, t, isum):
    if engine == "act":
        nc.scalar.mul(out, t, isum)
    elif engine == "dve":
        nc.vector.tensor_scalar_mul(out, t, isum)
    else:
        nc.gpsimd.tensor_scalar_mul(out, t, isum)


def build_nc():
    nc = bacc.Bacc()

    x = nc.dram_tensor("x", [BPC, M, D], F32, kind="ExternalInput")
    w1d, b1d, w2d, b2d = {}, {}, {}, {}
    for h in ("q", "k"):
        w1d[h] = nc.dram_tensor(f"{h}W1", [D, H], F32, kind="ExternalInput")
        b1d[h] = nc.dram_tensor(f"{h}b1", [H], F32, kind="ExternalInput")
        w2d[h] = nc.dram_tensor(f"{h}W2", [H, KF], F32, kind="ExternalInput")
        b2d[h] = nc.dram_tensor(f"{h}b2", [KF], F32, kind="ExternalInput")
    out = nc.dram_tensor("out", [BPC, M, M], F32, kind="ExternalOutput")

    # [b, p, n, d]: token (n*128+p), feature d
    x_r = x[:].rearrange("b (n p) d -> b p n d", p=128)
    # [b, p, n, m]: out[b, n*128+p, m]
    out_r = out[:].rearrange("b (n p) m -> b p n m", p=128)

    with tile.TileContext(nc) as tc:
        with (
            tc.tile_pool(name="consts", bufs=1) as consts,
            tc.tile_pool(name="xin", bufs=2) as xin_pool,
            tc.tile_pool(name="xt", bufs=2) as xt_pool,
            tc.tile_pool(name="ht", bufs=2) as ht_pool,
            tc.tile_pool(name="qkt", bufs=2) as qkt_pool,
            tc.tile_pool(name="texp", bufs=3) as t_pool,
            tc.tile_pool(name="osb", bufs=3) as out_pool,
            tc.tile_pool(name="small", bufs=6) as small_pool,
            tc.tile_pool(name="psum", bufs=2, space="PSUM") as psum_pool,
        ):
            # ---- constants ----
            ident = consts.tile([128, 128], BF16, tag="ident")
            make_identity(nc, ident)

            w1 = {}
            w2 = {}
            b1 = {}
            b2 = {}
            for h in ("q", "k"):
                w1[h] = consts.tile([D, H], BF16, tag=f"w1{h}", name=f"w1{h}")
                nc.gpsimd.dma_start(out=w1[h], in_=w1d[h][:])  # cast f32->bf16
                # [p, c, k]: W2[c*128+p, k]
                w2[h] = consts.tile([128, 2, KF], BF16, tag=f"w2{h}", name=f"w2{h}")
                nc.gpsimd.dma_start(
                    out=w2[h], in_=w2d[h][:].rearrange("(c p) k -> p c k", p=128)
                )
                b1[h] = consts.tile([128, 2], F32, tag=f"b1{h}", name=f"b1{h}")
                nc.gpsimd.dma_start(
                    out=b1[h], in_=b1d[h][:].rearrange("(c p) -> p c", p=128)
                )
                b2[h] = consts.tile([KF, 1], F32, tag=f"b2{h}", name=f"b2{h}")
                nc.gpsimd.dma_start(
                    out=b2[h], in_=b2d[h][:].rearrange("(k o) -> k o", o=1)
                )

            norm_i = 0
            for b in range(BPC):
                # ---- load x (HWDGE, plain f32), cast to bf16 on DVE ----
                xf = xin_pool.tile([128, MT, 128], F32, tag="xf", name="xf")
                nc.sync.dma_start(out=xf, in_=x_r[b])
                xsb = xin_pool.tile([128, MT, 128], BF16, tag="x")
                nc.vector.tensor_copy(xsb, xf)

                # ---- transpose x -> xT [d, token] via PE ----
                xT = xt_pool.tile([128, M], BF16, tag="xt")
                for g in range(2):
                    tp = psum_pool.tile([128, 1024], BF16, tag="ps")
                    for it in range(8):
                        nc.tensor.transpose(
                            tp[:, ts(it, 128)], xsb[:, g * 8 + it, :], ident
                        )
                    if XT_EVAC == "dve":
                        nc.vector.tensor_copy(xT[:, ts(g, 1024)], tp)
                    else:
                        nc.scalar.copy(xT[:, ts(g, 1024)], tp)

                # ---- MLP1: hT[h] [2, 128, M] = relu(W1.T @ xT + b1) ----
                ht = {}
                ri = 0
                for h in ("q", "k"):
                    ht[h] = ht_pool.tile([128, 2, M], BF16, tag=f"ht{h}", name=f"ht{h}")
                    for pc in range(2):
                        ps1 = psum_pool.tile([128, M], F32, tag="ps")
                        for fc in range(FC):
                            nc.tensor.matmul(
                                ps1[:, ts(fc, 512)],
                                lhsT=w1[h][:, ts(pc, 128)],
                                rhs=xT[:, ts(fc, 512)],
                                start=True,
                                stop=True,
                            )
                        _evac_bias(
                            nc,
                            RELU_ENGINES[ri % len(RELU_ENGINES)],
                            ht[h][:, pc, :],
                            ps1,
                            b1[h][:, pc : pc + 1],
                            relu=True,
                        )
                        ri += 1

                # ---- MLP2: QT/KT [KF, M] = W2.T @ hT + b2 ----
                qkt = {}
                for h in ("q", "k"):
                    ps2 = psum_pool.tile([KF, M], F32, tag="ps")
                    for fc in range(FC):
                        for kc in range(2):
                            nc.tensor.matmul(
                                ps2[:, ts(fc, 512)],
                                lhsT=w2[h][:, kc, :],
                                rhs=ht[h][:, kc, ts(fc, 512)],
                                start=(kc == 0),
                                stop=(kc == 1),
                            )
                    qkt[h] = qkt_pool.tile([KF, M], BF16, tag=f"qkt{h}", name=f"qkt{h}")
                    _evac_bias(
                        nc,
                        QT_EVAC if h == "q" else KT_EVAC,
                        qkt[h],
                        ps2,
                        b2[h],
                        relu=False,
                    )

                # ---- S + softmax per 128-row tile ----
                # Software-pipelined by one tile: the isum-reciprocal, norm,
                # and output DMA of tile rt-1 are emitted AFTER tile rt's
                # reduce/exp, so the in-order DVE/ACT queues never stall on
                # the exp(rt-1) -> isum(rt-1) -> norm(rt-1) tail.
                osb_tiles = {}
                pending = None

                def finish(j, t_j, ssum_j):
                    nonlocal norm_i
                    isum = small_pool.tile([128, 1], F32, tag="is", name="isum")
                    nc.vector.reciprocal(isum, ssum_j)
                    _norm(
                        nc,
                        NORM_PATTERN[norm_i % len(NORM_PATTERN)],
                        osb_tiles[j // PAIR][:, ts(j % PAIR, M)],
                        t_j,
                        isum,
                    )
                    norm_i += 1
                    if j % PAIR == PAIR - 1:
                        nc.sync.dma_start(
                            out=out_r[b][:, j - PAIR + 1 : j + 1, :],
                            in_=osb_tiles.pop(j // PAIR),
                        )

                for rt in range(MT):
                    ps_s = psum_pool.tile([128, M], F32, tag="ps")
                    # chunked row-max: reduce each 512-col bank as soon as its
                    # matmul lands, so the reduce overlaps the remaining
                    # matmuls instead of serializing after all four.
                    m4 = small_pool.tile([128, FC], F32, tag="m4", name="m4")
                    for fc in range(FC):
                        nc.tensor.matmul(
                            ps_s[:, ts(fc, 512)],
                            lhsT=qkt["q"][:, ts(rt, 128)],
                            rhs=qkt["k"][:, ts(fc, 512)],
                            start=True,
                            stop=True,
                        )
                        nc.vector.reduce_max(
                            m4[:, fc : fc + 1],
                            ps_s[:, ts(fc, 512)],
                            axis=mybir.AxisListType.X,
                        )

                    m_t = small_pool.tile([128, 1], F32, tag="m")
                    nc.vector.reduce_max(m_t, m4, axis=mybir.AxisListType.X)
                    im = small_pool.tile([128, 1], F32, tag="im")
                    nc.vector.reciprocal(im, m_t)

                    t_t = t_pool.tile([128, M], F32, tag="t")
                    ssum = small_pool.tile([128, 1], F32, tag="ss")
                    nc.scalar.activation(
                        t_t, ps_s, AF.Exp, bias=0.0, scale=im, accum_out=ssum
                    )

                    if rt % PAIR == 0:
                        osb_tiles[rt // PAIR] = out_pool.tile(
                            [128, PAIR * M], F32, tag="o", name="osb"
                        )
                    if pending is not None:
                        finish(*pending)
                    pending = (rt, t_t, ssum)
                finish(*pending)
    nc.finalize()
    return nc


_NC_CACHE = None


def _get_nc():
    global _NC_CACHE
    if _NC_CACHE is None:
        _NC_CACHE = build_nc()
    return _NC_CACHE


def run(inputs, trace=False, trace_cores=None):
    """Run on 8 cores; returns (full_output [B,M,M] f32, BassKernelResults)."""
    nc = _get_nc()
    in_maps = []
    x = np.ascontiguousarray(inputs["x"], dtype=np.float32)
    for c in range(N_CORES):
        im = {"x": np.ascontiguousarray(x[c * BPC : (c + 1) * BPC])}
        for k in ("qW1", "qb1", "qW2", "qb2", "kW1", "kb1", "kW2", "kb2"):
            im[k] = np.ascontiguousarray(inputs[k], dtype=np.float32)
        in_maps.append(im)
    res = run_bass_kernel_spmd(
        nc,
        in_maps,
        core_ids=list(range(N_CORES)),
        trace=trace,
        trace_cores=trace_cores,
    )
    outs = [r["out"] for r in res.results]
    full = np.concatenate(outs, axis=0)
    assert full.shape == (B, M, M) and full.dtype == np.float32
    return full, res


def kernel(**inputs) -> np.ndarray:
    out, _ = run(inputs, trace=False)
    return out


# revision 10
# speedup vs baseline: 1.1905x; 1.1905x over previous
"""Self-contained Trainium2 Bass kernel for nn_CA_9363028705415 (sparse_attention).

Computes, per batch b:
    Q = relu(x[b] @ qW1 + qb1) @ qW2 + qb2          # [M, K]
    Kt = relu(x[b] @ kW1 + kb1) @ kW2 + kb2         # [M, K]
    S = Q @ Kt.T                                    # [M, M]
    out[b] = softmax(S / rowmax(S), axis=-1)        # max-DIVISION normalization

Shapes: B=16, M=2048, D=128, H=256, K=64.  Output [16, 2048, 2048] f32 (256 MB)
=> memory-bound on the output write (~32 MB/core across 8 cores).

Sharding: data-parallel over batch across 8 NeuronCores; 2 batches/core; tiny
MLP weights replicated.  Single NEFF run SPMD via run_bass_kernel_spmd.
"""

import numpy as np

import concourse.bass as bass
import concourse.mybir as mybir
from concourse import bacc
import concourse.tile as tile
from concourse.bass import ts
from concourse.bass_utils import run_bass_kernel_spmd
from concourse.masks import make_identity

F32 = mybir.dt.float32
BF16 = mybir.dt.bfloat16
FP16 = mybir.dt.float16
AF = mybir.ActivationFunctionType
ALU = mybir.AluOpType

N_CORES = 8
B, M, D, H, KF = 16, 2048, 128, 256, 64
BPC = B // N_CORES     # batches per core
MT = M // 128          # 16 row-tiles per batch
FC = M // 512          # 4 matmul free-chunks of 512
PAIR = 2               # row-tiles per output DMA (2 MB chunks)

# Engine assignment knobs (tuned from profiling).
# normalize engine per row-tile, cycled: DVE tensor_scalar is 2x fp32 SBUF
# (1.28us/tile), ACT copy-with-scale is 1x (2.06us/tile).  gpsimd is NOT used:
# its software tensor_scalar is ~29us/tile AND locks the DVE-shared SBUF port.
NORM_PATTERN = (
    "dve", "act", "dve", "act", "dve", "dve", "dve", "act",
    "dve", "act", "dve", "dve", "dve", "act", "dve", "dve",
)
RELU_ENGINES = ("act", "act", "act", "act")  # hT evac per (head, pc)
QT_EVAC = "dve"
KT_EVAC = "act"
XT_EVAC = "dve"


def _evac_bias(nc, engine, out, in_, bias, relu):
    """out = [relu](in_ + bias), bias is [P,1] per-partition AP."""
    if engine == "act":
        nc.scalar.activation(
            out, in_, AF.Relu if relu else AF.Identity, bias=bias, scale=1.0
        )
    else:
        eng = nc.vector if engine == "dve" else nc.gpsimd
        if relu:
            eng.tensor_scalar(
                out, in_, bias, 0.0, op0=ALU.add, op1=ALU.max
            )
        else:
            eng.tensor_scalar(out, in_, bias, None, op0=ALU.add)


def _norm(nc, engine, out, t, isum):
    if engine == "act":
        nc.scalar.mul(out, t, isum)
    elif engine == "dve":
        nc.vector.tensor_scalar_mul(out, t, isum)
    else:
        nc.gpsimd.tensor_scalar_mul(out, t, isum)


def build_nc():
    nc = bacc.Bacc()

    x = nc.dram_tensor("x", [BPC, M, D], F32, kind="ExternalInput")
    w1d, b1d, w2d, b2d = {}, {}, {}, {}
    for h in ("q", "k"):
        w1d[h] = nc.dram_tensor(f"{h}W1", [D, H], F32, kind="ExternalInput")
        b1d[h] = nc.dram_tensor(f"{h}b1", [H], F32, kind="ExternalInput")
        w2d[h] = nc.dram_tensor(f"{h}W2", [H, KF], F32, kind="ExternalInput")
        b2d[h] = nc.dram_tensor(f"{h}b2", [KF], F32, kind="ExternalInput")
    out = nc.dram_tensor("out", [BPC, M, M], F32, kind="ExternalOutput")

    # [b, p, n, d]: token (n*128+p), feature d
    x_r = x[:].rearrange("b (n p) d -> b p n d", p=128)
    # [b, p, n, m]: out[b, n*128+p, m]
    out_r = out[:].rearrange("b (n p) m -> b p n m", p=128)

    with tile.TileContext(nc) as tc:
        with (
            tc.tile_pool(name="consts", bufs=1) as consts,
            tc.tile_pool(name="xin", bufs=2) as xin_pool,
            tc.tile_pool(name="xt", bufs=2) as xt_pool,
            tc.tile_pool(name="ht", bufs=2) as ht_pool,
            tc.tile_pool(name="qkt", bufs=2) as qkt_pool,
            tc.tile_pool(name="texp", bufs=3) as t_pool,
            tc.tile_pool(name="osb", bufs=3) as out_pool,
            tc.tile_pool(name="small", bufs=6) as small_pool,
            tc.tile_pool(name="psum", bufs=2, space="PSUM") as psum_pool,
        ):
            # ---- constants ----
            ident = consts.tile([128, 128], BF16, tag="ident")
            make_identity(nc, ident)

            w1 = {}
            w2 = {}
            b1 = {}
            b2 = {}
            for h in ("q", "k"):
                w1[h] = consts.tile([D, H], BF16, tag=f"w1{h}", name=f"w1{h}")
                nc.gpsimd.dma_start(out=w1[h], in_=w1d[h][:])  # cast f32->bf16
                # [p, c, k]: W2[c*128+p, k]
                w2[h] = consts.tile([128, 2, KF], BF16, tag=f"w2{h}", name=f"w2{h}")
                nc.gpsimd.dma_start(
                    out=w2[h], in_=w2d[h][:].rearrange("(c p) k -> p c k", p=128)
                )
                b1[h] = consts.tile([128, 2], F32, tag=f"b1{h}", name=f"b1{h}")
                nc.gpsimd.dma_start(
                    out=b1[h], in_=b1d[h][:].rearrange("(c p) -> p c", p=128)
                )
                b2[h] = consts.tile([KF, 1], F32, tag=f"b2{h}", name=f"b2{h}")
                nc.gpsimd.dma_start(
                    out=b2[h], in_=b2d[h][:].rearrange("(k o) -> k o", o=1)
                )

            norm_i = 0
            for b in range(BPC):
                # ---- load x (HWDGE, plain f32), cast to bf16 on DVE ----
                xf = xin_pool.tile([128, MT, 128], F32, tag="xf", name="xf")
                nc.sync.dma_start(out=xf, in_=x_r[b])
                xsb = xin_pool.tile([128, MT, 128], BF16, tag="x")
                nc.vector.tensor_copy(xsb, xf)

                # ---- transpose x -> xT [d, token] via PE ----
                xT = xt_pool.tile([128, M], BF16, tag="xt")
                for g in range(2):
                    tp = psum_pool.tile([128, 1024], BF16, tag="ps")
                    for it in range(8):
                        nc.tensor.transpose(
                            tp[:, ts(it, 128)], xsb[:, g * 8 + it, :], ident
                        )
                    if XT_EVAC == "dve":
                        nc.vector.tensor_copy(xT[:, ts(g, 1024)], tp)
                    else:
                        nc.scalar.copy(xT[:, ts(g, 1024)], tp)

                # ---- MLP1: hT[h] [2, 128, M] = relu(W1.T @ xT + b1) ----
                ht = {}
                ri = 0
                for h in ("q", "k"):
                    ht[h] = ht_pool.tile([128, 2, M], BF16, tag=f"ht{h}", name=f"ht{h}")
                    for pc in range(2):
                        ps1 = psum_pool.tile([128, M], F32, tag="ps")
                        for fc in range(FC):
                            nc.tensor.matmul(
                                ps1[:, ts(fc, 512)],
                                lhsT=w1[h][:, ts(pc, 128)],
                                rhs=xT[:, ts(fc, 512)],
                                start=True,
                                stop=True,
                            )
                        _evac_bias(
                            nc,
                            RELU_ENGINES[ri % len(RELU_ENGINES)],
                            ht[h][:, pc, :],
                            ps1,
                            b1[h][:, pc : pc + 1],
                            relu=True,
                        )
                        ri += 1

                # ---- MLP2: QT/KT [KF, M] = W2.T @ hT + b2 ----
                qkt = {}
                for h in ("q", "k"):
                    ps2 = psum_pool.tile([KF, M], F32, tag="ps")
                    for fc in range(FC):
                        for kc in range(2):
                            nc.tensor.matmul(
                                ps2[:, ts(fc, 512)],
                                lhsT=w2[h][:, kc, :],
                                rhs=ht[h][:, kc, ts(fc, 512)],
                                start=(kc == 0),
                                stop=(kc == 1),
                            )
                    qkt[h] = qkt_pool.tile([KF, M], BF16, tag=f"qkt{h}", name=f"qkt{h}")
                    _evac_bias(
                        nc,
                        QT_EVAC if h == "q" else KT_EVAC,
                        qkt[h],
                        ps2,
                        b2[h],
                        relu=False,
                    )

                # ---- S + softmax per 128-row tile ----
                # Software-pipelined by one tile: the isum-reciprocal, norm,
                # and output DMA of tile rt-1 are emitted AFTER tile rt's
                # reduce/exp, so the in-order DVE/ACT queues never stall on
                # the exp(rt-1) -> isum(rt-1) -> norm(rt-1) tail.
                osb_tiles = {}
                pending = None

                def finish(j, t_j, ssum_j):
                    nonlocal norm_i
                    isum = small_pool.tile([128, 1], F32, tag="is", name="isum")
                    nc.vector.reciprocal(isum, ssum_j)
                    _norm(
                        nc,
                        NORM_PATTERN[norm_i % len(NORM_PATTERN)],
                        osb_tiles[j // PAIR][:, ts(j % PAIR, M)],
                        t_j,
                        isum,
                    )
                    norm_i += 1
                    if j % PAIR == PAIR - 1:
                        nc.sync.dma_start(
                            out=out_r[b][:, j - PAIR + 1 : j + 1, :],
                            in_=osb_tiles.pop(j // PAIR),
                        )

                for rt in range(MT):
                    ps_s = psum_pool.tile([128, M], F32, tag="ps")
                    # Evacuate S from PSUM to fp16 SBUF with a fused row-max
                    # (tensor_scalar accum_out reduces with op1).  Two
                    # 1024-wide chunks so the first copy overlaps the last
                    # matmuls; the PSUM slot frees right after the copies,
                    # and exp reads the SBUF copy instead of PSUM.
                    sc_t = t_pool.tile([128, M], FP16, tag="sc", name="sc")
                    m2 = small_pool.tile([128, 2], F32, tag="m2", name="m2")
                    for fc in range(FC):
                        nc.tensor.matmul(
                            ps_s[:, ts(fc, 512)],
                            lhsT=qkt["q"][:, ts(rt, 128)],
                            rhs=qkt["k"][:, ts(fc, 512)],
                            start=True,
                            stop=True,
                        )
                        if fc % 2 == 1:
                            h = fc // 2
                            nc.vector.tensor_scalar(
                                sc_t[:, ts(h, 1024)],
                                ps_s[:, ts(h, 1024)],
                                0.0,
                                None,
                                op0=ALU.add,
                                op1=ALU.max,
                                accum_out=m2[:, h : h + 1],
                            )

                    m_t = small_pool.tile([128, 1], F32, tag="m")
                    nc.vector.reduce_max(m_t, m2, axis=mybir.AxisListType.X)
                    im = small_pool.tile([128, 1], F32, tag="im")
                    nc.vector.reciprocal(im, m_t)

                    t_t = t_pool.tile([128, M], F32, tag="t")
                    ssum = small_pool.tile([128, 1], F32, tag="ss")
                    nc.scalar.activation(
                        t_t, sc_t, AF.Exp, bias=0.0, scale=im, accum_out=ssum
                    )

                    if rt % PAIR == 0:
                        osb_tiles[rt // PAIR] = out_pool.tile(
                            [128, PAIR * M], F32, tag="o", name="osb"
                        )
                    if pending is not None:
                        finish(*pending)
                    pending = (rt, t_t, ssum)
                finish(*pending)
    nc.finalize()
    return nc


_NC_CACHE = None


def _get_nc():
    global _NC_CACHE
    if _NC_CACHE is None:
        _NC_CACHE = build_nc()
    return _NC_CACHE


def run(inputs, trace=False, trace_cores=None):
    """Run on 8 cores; returns (full_output [B,M,M] f32, BassKernelResults)."""
    nc = _get_nc()
    in_maps = []
    x = np.ascontiguousarray(inputs["x"], dtype=np.float32)
    for c in range(N_CORES):
        im = {"x": np.ascontiguousarray(x[c * BPC : (c + 1) * BPC])}
        for k in ("qW1", "qb1", "qW2", "qb2", "kW1", "kb1", "kW2", "kb2"):
            im[k] = np.ascontiguousarray(inputs[k], dtype=np.float32)
        in_maps.append(im)
    res = run_bass_kernel_spmd(
        nc,
        in_maps,
        core_ids=list(range(N_CORES)),
        trace=trace,
        trace_cores=trace_cores,
    )
    outs = [r["out"] for r in res.results]
    full = np.concatenate(outs, axis=0)
    assert full.shape == (B, M, M) and full.dtype == np.float32
    return full, res


def kernel(**inputs) -> np.ndarray:
    out, _ = run(inputs, trace=False)
    return out
